# revision 21
# baseline (speedup 1.0000x reference)
"""Trainium2 Bass kernel for nn_Encoder_61770219651232 (dual-quaternion skinning).

Computation per node n (N = 2,000,000):
    qs = W[n, :10] @ qm4            (qm4 = x.reshape(10, 4), shared)
    q  = qs / |qs|                  (normalize)
    y3 = R(q) @ v                   (rotate v = VR[4n:4n+3])
    y  = [y3, r]                    (r = VR[4n+3] passes through)

Strategy (pure data parallel over nodes, 8 cores):
  - All HBM tensors are fp16, restaged on the host: W is pre-transposed into
    per-megablock (120, 2048) tiles so no PE transpose is needed; VR is
    deinterleaved into planar x/y/z planes (r passes through on the host);
    y is written back as 3 planar fp16 planes.  ~9.7MB HBM traffic per core.
  - Blend: 16 matmuls per megablock (stationary = (120,128) W slice, moving
    = (120,48) block-diag qm4, plane-major columns) into one (128,1024)
    fp32 PSUM tile (one bank per half); one ACT cast-copy per half lands q
    as planar fp16 SBUF planes.
  - Rotate, planar fp16 (all packed 2-byte ops -> DVE 2x mode), with the
    normalization folded in early to shorten the critical chain:
        g  = 2/|q|^2, gw = g*qw, qg = g*qv
        t  = qv x v            (DVE)
        P1 = v + gw*t          (Pool, off the critical chain)
        y  = P1 + qg x t       (DVE)
    Pool ops are issued as scalar_tensor_tensor (cheaper gpsimd path).
  - Software-pipelined emission: per-granule phases are interleaved
    (ph0/ph1 run one granule ahead of ph2) so no engine stream stalls on a
    cross-engine dependency; all loads are emitted before any store on the
    sync queue.
Scale-relative error vs the fp32 jax reference: ~1e-3 (fp16 staging).
"""
import sys

sys.path.insert(0, "/opt/trn_rl_repo")

import numpy as np

N_NODES = 2_000_000
N_CORES = 8
MB_NODES = 24576          # nodes per megablock (128 x 192)
NMB = 11                  # megablocks per core
NPC = MB_NODES * NMB      # 270336 nodes per core
N_PAD = NPC * N_CORES     # 2162688 padded total
GRANULES = [(0, 1), (1, 2), (3, 3), (6, 3), (9, 2)]  # (first mb, num mbs)
GN_MAX = 3

_compiled = None


def _build_kernel():
    import concourse.bacc as bacc
    import concourse.tile as tile
    from concourse import mybir

    f32 = mybir.dt.float32
    f16 = mybir.dt.float16
    Alu = mybir.AluOpType
    Act = mybir.ActivationFunctionType

    nc = bacc.Bacc("TRN2", target_bir_lowering=False, debug=False,
                   num_devices=N_CORES)

    wt_dram = nc.dram_tensor("wt", [NMB * 120 * 2048], f16, kind="ExternalInput")
    v_dram = nc.dram_tensor("v3", [NMB * 128 * 576], f16, kind="ExternalInput")
    bd_dram = nc.dram_tensor("bd", [120, 48], f16, kind="ExternalInput")
    y_dram = nc.dram_tensor("y", [NMB * 128 * 576], f16, kind="ExternalOutput")

    wt3 = wt_dram.ap().rearrange("(m p e) -> m p e", m=NMB, p=120)   # e=2048
    v4 = v_dram.ap().rearrange("(m f i e) -> f m i e", m=NMB, f=128, i=3)
    y4 = y_dram.ap().rearrange("(m f i e) -> f m i e", m=NMB, f=128, i=3)

    from contextlib import ExitStack

    with tile.TileContext(nc) as tc, ExitStack() as ctx:
        consts = ctx.enter_context(tc.tile_pool(name="consts", bufs=1))
        wpool = ctx.enter_context(tc.tile_pool(name="wpool", bufs=4))
        vpool = ctx.enter_context(tc.tile_pool(name="vpool", bufs=4))
        qpool = ctx.enter_context(tc.tile_pool(name="qpool", bufs=2))
        spool = ctx.enter_context(tc.tile_pool(name="spool", bufs=2))
        pspool = ctx.enter_context(tc.tile_pool(name="pspool", bufs=3, space="PSUM"))

        bd_sb = consts.tile([120, 48], f16)
        nc.sync.dma_start(out=bd_sb[:], in_=bd_dram.ap())

        SQ_SCALE = 16.0  # keeps |q|^2 squares out of fp16 subnormals
        state = {}

        def pool_tt(out, a, b, op):
            if op == Alu.mult:
                nc.gpsimd.tensor_mul(out, a, b)
            else:
                nc.gpsimd.tensor_add(out, a, b)

        def ph0(gi):
            """Loads, blend matmuls, q cast-copies + squares for granule gi."""
            g0, gn = GRANULES[gi]
            q16 = qpool.tile([128, GN_MAX, 4, 192], f16, tag="q", name="q16")[:, :gn]
            S_full = spool.tile([128, GN_MAX, 4, 192], f16, tag="S", name="S")
            S = S_full[:, :gn]
            v_t = vpool.tile([128, GN_MAX, 3, 192], f16, tag="v", name="v_t")[:, :gn]
            wts = []
            for k in range(gn):
                wt_t = wpool.tile([120, 2048], f16, tag="wt")
                nc.sync.dma_start(out=wt_t[:], in_=wt3[g0 + k])
                wts.append(wt_t)
                if k == 0:
                    nc.sync.dma_start(out=v_t[:], in_=v4[:, g0:g0 + gn])
            for k in range(gn):
                ps = pspool.tile([128, 1024], f32, tag="ps")
                for h in range(2):
                    for c8 in range(8):
                        cc = 8 * h + c8
                        nc.tensor.matmul(
                            ps[:, 512 * h + 48 * c8: 512 * h + 48 * c8 + 48],
                            wts[k][:, 128 * cc:128 * cc + 128],
                            bd_sb[:],
                        )
                for h in range(2):
                    src = ps[:, 512 * h:512 * h + 384].rearrange(
                        "p (c j s) -> p j c s", j=4, s=12)
                    dst = q16[:, k, :, 96 * h:96 * h + 96].rearrange(
                        "p j (c s) -> p j c s", s=12)
                    nc.scalar.copy(out=dst, in_=src)
                # per-mb |q_j|^2/2 squares so S(g) is ready right after the
                # last copy (scale folds the 2 of g = 2/|q|^2)
                nc.scalar.activation(S[:, k], q16[:, k], Act.Square, scale=SQ_SCALE)
            state[gi] = {"v": v_t, "q": q16, "S": S, "gn": gn, "g0": g0}

        def ph1(gi):
            """First cross product + norm chain + T' = g*T for granule gi."""
            st = state[gi]
            gn, q16, v_t, S = st["gn"], st["q"], st["v"], st["S"]

            def pl(tag, w=1, dt=f16):
                return spool.tile([128, GN_MAX, w, 192], dt, tag=tag,
                                  name=tag)[:, :gn]

            Q = [q16[:, :, j, :] for j in range(4)]
            V = [v_t[:, :, i, :] for i in range(3)]

            # t = qv x v, all on DVE (Pool's slower ops stay off this chain)
            A = pl("A", 3)
            B = pl("B", 3)
            T = pl("T", 3)
            nc.vector.tensor_mul(A[:, :, 0, :], Q[1], V[2])
            nc.vector.tensor_mul(A[:, :, 1, :], Q[2], V[0])
            nc.vector.tensor_mul(A[:, :, 2, :], Q[0], V[1])
            nc.vector.tensor_mul(B[:, :, 0, :], Q[2], V[1])
            nc.vector.tensor_mul(B[:, :, 1, :], Q[0], V[2])
            nc.vector.tensor_mul(B[:, :, 2, :], Q[1], V[0])

            # norm chain: N2 (DVE), n32 (Pool), recip (DVE), cast (ACT).
            # T-sub sits between recip's wait targets so DVE doesn't stall.
            N2 = pl("N2", 2)
            nc.vector.tensor_add(N2[:], S[:, :, 0:2, :], S[:, :, 2:4, :])
            n32 = pl("n32", 1, f32)
            nc.vector.tensor_add(n32[:, :, 0, :], N2[:, :, 0, :], N2[:, :, 1, :])
            nc.vector.tensor_sub(T[:], A[:], B[:])
            # wt = qw*t on Pool, ready as soon as T lands (no g dependency)
            WT = pl("WT", 3)
            for i in range(3):
                pool_tt(WT[:, :, i, :], q16[:, :, 3, :], T[:, :, i, :],
                         Alu.mult)
            g32 = pl("g32", 1, f32)
            nc.vector.reciprocal_approx_fast(out=g32[:, :, 0, :],
                                             in_=n32[:, :, 0, :])
            g16 = pl("g16", 1)
            nc.scalar.mul(out=g16[:], in_=g32[:], mul=2.0 * SQ_SCALE * SQ_SCALE)
            st.update(A=A, B=B, T=T, WT=WT, G=g16[:, :, 0, :])

        def ph2(gi):
            """c = qv x t (DVE/Pool), u = c + wt, e = g*u, y = v + e (Pool)."""
            st = state[gi]
            gn, g0, v_t, q16 = st["gn"], st["g0"], st["v"], st["q"]
            A, B, T, WT, G = st["A"], st["B"], st["T"], st["WT"], st["G"]
            Q = [q16[:, :, j, :] for j in range(4)]
            Tv = [T[:, :, i, :] for i in range(3)]

            def pl(tag, w=1, dt=f16):
                return spool.tile([128, GN_MAX, w, 192], dt, tag=tag,
                                  name=tag)[:, :gn]

            # c = qv x t  (reuse A/B; Pool takes two B-muls)
            nc.vector.tensor_mul(A[:, :, 0, :], Q[1], Tv[2])
            nc.vector.tensor_mul(A[:, :, 1, :], Q[2], Tv[0])
            nc.vector.tensor_mul(A[:, :, 2, :], Q[0], Tv[1])
            pool_tt(B[:, :, 0, :], Q[2], Tv[1], Alu.mult)
            pool_tt(B[:, :, 1, :], Q[0], Tv[2], Alu.mult)
            nc.vector.tensor_mul(B[:, :, 2, :], Q[1], Tv[0])
            C = pl("C", 3)
            nc.vector.tensor_sub(C[:], A[:], B[:])
            # u = c + wt; e = g*u; y = v + e (Pool fat add gates only the
            # store, which has DMA slack; DVE takes the last granule's y
            # to shorten the drain tail)
            nc.vector.tensor_add(C[:], C[:], WT[:])
            for i in range(3):
                nc.vector.tensor_mul(C[:, :, i, :], C[:, :, i, :], G)
            if gi == len(GRANULES) - 1:
                nc.vector.tensor_add(v_t[:], C[:], v_t[:])
            else:
                pool_tt(v_t[:], C[:], v_t[:], Alu.add)
            st["done"] = True

        def store(gi):
            st = state[gi]
            g0, gn = st["g0"], st["gn"]
            nc.sync.dma_start(out=y4[:, g0:g0 + gn], in_=st["v"])

        # software pipeline: ph1 runs one granule ahead of ph2 so every
        # cross-engine dependency has a full phase of slack
        ph0(0); ph0(1)
        ph1(0)
        ph0(2)
        ph1(1); ph2(0)
        ph0(3)
        ph1(2); ph2(1)
        ph0(4)
        ph1(3); store(0); ph2(2)
        ph1(4); store(1); ph2(3)
        store(2); ph2(4)
        store(3); store(4)

    nc.compile()
    return nc


def _get_compiled():
    global _compiled
    if _compiled is None:
        _compiled = _build_kernel()
    return _compiled


def _stage_inputs(x, weights, VR):
    """Host-side restaging: pad, shard, pre-transpose W, deinterleave VR."""
    qm4 = x.reshape(10, 4)
    bd = np.zeros((120, 48), np.float32)
    for s in range(12):
        bd[10 * s:10 * s + 10, s::12] = qm4
    bd = bd.astype(np.float16)

    w_pad = np.zeros((N_PAD, 10), np.float32)
    w_pad[:N_NODES] = weights
    # (core, mb, f, cc, s, k) -> (core, mb, s, k, cc, f)
    wt = w_pad.reshape(N_CORES, NMB, 128, 16, 12, 10)
    wt = np.ascontiguousarray(wt.transpose(0, 1, 4, 5, 3, 2)).astype(np.float16)
    wt = wt.reshape(N_CORES, -1)

    vr_pad = np.zeros((N_PAD, 4), np.float32)
    vr_pad[:N_NODES] = VR.reshape(-1, 4)
    # (core, mb, f, m, comp) -> planar (core, mb, f, comp, m)
    v3 = vr_pad.reshape(N_CORES, NMB, 128, 192, 4)[..., :3]
    v3 = np.ascontiguousarray(v3.transpose(0, 1, 2, 4, 3)).astype(np.float16)
    v3 = v3.reshape(N_CORES, -1)
    return bd, wt, v3, vr_pad


def kernel(x, weights, VR):
    from concourse import bass_utils

    x = np.asarray(x, dtype=np.float32)
    weights = np.asarray(weights, dtype=np.float32)
    VR = np.asarray(VR, dtype=np.float32)

    bd, wt, v3, vr_pad = _stage_inputs(x, weights, VR)

    nc = _get_compiled()
    in_maps = [
        {"wt": wt[c], "v3": v3[c], "bd": bd}
        for c in range(N_CORES)
    ]
    res = bass_utils.run_bass_kernel_spmd(nc, in_maps, core_ids=list(range(N_CORES)))

    out = np.empty((N_PAD, 4), np.float32)
    for c in range(N_CORES):
        yc = res.results[c]["y"].reshape(NMB, 128, 3, 192)
        yc = yc.transpose(0, 1, 3, 2).reshape(NPC, 3)
        out[c * NPC:(c + 1) * NPC, :3] = yc.astype(np.float32)
    out[:, 3] = vr_pad[:, 3]
    return out[:N_NODES].reshape(-1)


if __name__ == "__main__":
    rng = np.random.default_rng(0)
    x = rng.standard_normal(40).astype(np.float32)
    W = (rng.standard_normal((N_NODES, 10)) * 0.1).astype(np.float32)
    VR = rng.standard_normal(N_NODES * 4).astype(np.float32)
    y = kernel(x, weights=W, VR=VR)
    print("kernel ran, y shape", y.shape, y[:8])
